# revision 29
# baseline (speedup 1.0000x reference)
"""Trainium2 Bass kernel for nn_CapsLayer (capsule routing layer).

Problem (hardcoded): B=32, N=8192, P=8, J=16, D=16, R=3 routing iters.
  u_hat = einsum('jnpd,bnp->bjnd', w, u)      (never materialized!)
  R iters: c = softmax(b, axis=n); s = einsum('jn,bjnd->bjd', c, u_hat)
           v = squash(s); b += mean_b einsum('bjnd,bjd->bjn', u_hat, v)

Sharding: J (16 output caps) split across 8 cores, 2 caps/core ("jl" = local
j). Zero collectives. Each core receives the full u and its w[j-pair] slice,
host-pretransposed into SBUF-ready fp16 layouts so every load is one
contiguous [128, X] DMA.

Per-core algorithm (never materializes the 33MB u_hat):
  s-chain n-factorization: n = 128*h + q  (q = partition, h = 0..64)
  g-chain n-factorization: n = 2048*gc + k (gc = 0..4, k = 0..2048)

  s-pass (per iter): cw = exp(b) * w (one DVE mult with a broadcast AP);
    512 small matmuls contract (n,p) into one PSUM bank:
      s[b, (jl,d)] += usin[:, p, h, :].T @ cw[:, p, :, :, h]
    softmax's 1/Z is folded into s's evacuation.
  g-pass (per iter): rho[(gc,jl,d), (p,k)] = sum_b v[b,jl,d]*u[b,n,p] via a
    block-diag-v stationary matmul (moving = u7in, wide free dim); then
    prod = rho * w8in (DVE mult, also evacuates PSUM); then a selector
    matmul reduces the d partitions and folds 1/B:
      g[(jl,gc), k] += Sel.T @ prod   (PSUM-accumulated over p)
  b is relaid-out [8,2048] -> [128,(jl,h)] once per iter via a 16KB DRAM
  bounce + DMA xbar transpose; squash/softmax stats are tiny ops.
"""

import os
import sys

import numpy as np

B, N, P, J, D, R = 32, 8192, 8, 16, 16, 3
EPS = 1e-9
NCORES = 8
JL = J // NCORES  # 2 output caps per core
G = 4  # gc groups (n // 2048)
H = N // 128  # 64
K = N // G  # 2048
KC = 512  # free-dim chunk (one PSUM bank)

_prog_cache = {}


def _ensure_path():
    for p in ("/opt/trn_rl_repo", "/root/.axon_site/_ro/trn_rl_repo"):
        if os.path.isdir(p) and p not in sys.path:
            sys.path.insert(0, p)


def _build_program(variant="full"):
    """Build the SPMD bass/tile program (same program for all 8 cores)."""
    _ensure_path()
    import concourse.bass as bass
    import concourse.bacc as bacc
    import concourse.mybir as mybir
    import concourse.tile as tile

    f32 = mybir.dt.float32
    f16 = mybir.dt.float16
    AF = mybir.ActivationFunctionType
    ALU = mybir.AluOpType
    AX = mybir.AxisListType

    nc = bacc.Bacc("TRN2", target_bir_lowering=False, debug=False)

    # host-pretransposed fp16 inputs (see _prep_inputs for layouts)
    us_d = nc.dram_tensor("usin", [128, P, H, B], f16, kind="ExternalInput")
    ws_d = nc.dram_tensor("wsin", [128, P, JL, D, H], f16, kind="ExternalInput")
    f8 = mybir.dt.float8e4
    u7_d = nc.dram_tensor("u7in", [128, P, K], f8, kind="ExternalInput")
    w8_d = nc.dram_tensor("w8in", [128, P, K], f8, kind="ExternalInput")
    sel_d = nc.dram_tensor("selred", [128, 2 * G], f16, kind="ExternalInput")
    ones_d = nc.dram_tensor("ones128", [128, 1], f32, kind="ExternalInput")
    onesrow_d = nc.dram_tensor("onesrow", [1, B], f32, kind="ExternalInput")
    ident_d = nc.dram_tensor("ident8", [2 * G, 2 * G], f16, kind="ExternalInput")
    vout_d = nc.dram_tensor("vout", [B, JL, D], f32, kind="ExternalOutput")
    # 16KB fp16 bounce buffer for the b-logit relayout, laid out [jl, n]
    cb_d = nc.dram_tensor("cb", [JL, N], f16)

    with tile.TileContext(nc) as tc:
        with (
            tc.tile_pool(name="big", bufs=1) as big,
            tc.tile_pool(name="small", bufs=1) as small,
            tc.tile_pool(name="prod", bufs=4) as prodp,
            tc.tile_pool(name="rho_ps", bufs=2, space="PSUM") as rho_ps,
            tc.tile_pool(name="acc_ps", bufs=1, space="PSUM") as acc_ps,
            tc.tile_pool(name="tiny_ps", bufs=1, space="PSUM") as tiny_ps,
        ):
            # ---------------- persistent SBUF tiles ----------------
            usin = big.tile([128, P, H, B], f16, tag="usin")
            wsin = big.tile([128, P, JL, D, H], f16, tag="wsin")
            cw = big.tile([128, P, JL, D, H], f16, tag="cw")
            u7in = big.tile([128, P, K], f8, tag="u7in")
            w8in = big.tile([128, P, K], f8, tag="w8in")

            selred = small.tile([128, 2 * G], f16, tag="selred")
            ones128 = small.tile([128, 1], f32, tag="ones")
            onesrow = small.tile([1, B], f32, tag="onesrow")
            ident8 = small.tile([2 * G, 2 * G], f16, tag="ident8")
            vblk = small.tile([128, 128], f16, tag="vblk")  # block-diag v
            b_sb = small.tile([2 * G, K], f16, tag="b")  # [(jl,gc), k]
            braw = small.tile([128, JL, H], f16, tag="braw")
            cq = small.tile([128, JL, H], f16, tag="cq")  # exp(b)
            recipz = small.tile([B, JL], f32, tag="recipz")

            # ---------------- loads + init ----------------
            # 2.1MB chunks (s-chain tensors first; HWDGE is FIFO)
            hp = P // 2
            for h0 in range(2):
                sl = slice(h0 * hp, (h0 + 1) * hp)
                nc.sync.dma_start(out=usin[:, sl], in_=us_d.ap()[:, sl])
                nc.sync.dma_start(out=wsin[:, sl], in_=ws_d.ap()[:, sl])
            for h0 in range(2):
                sl = slice(h0 * hp, (h0 + 1) * hp)
                nc.sync.dma_start(out=u7in[:, sl], in_=u7_d.ap()[:, sl])
                nc.sync.dma_start(out=w8in[:, sl], in_=w8_d.ap()[:, sl])
            nc.sync.dma_start(out=selred[:], in_=sel_d.ap())
            nc.sync.dma_start(out=ones128[:], in_=ones_d.ap())
            nc.sync.dma_start(out=onesrow[:], in_=onesrow_d.ap())
            nc.sync.dma_start(out=ident8[:], in_=ident_d.ap())
            nc.gpsimd.memset(vblk[:], 0.0)
            nc.gpsimd.memset(b_sb[:], 0.0)

            # ---------------- helpers ----------------
            def s_pass(r, stat):
                """s_psum[b, (jl,d)] = sum_{q,p,h} usin.T @ stat."""
                s_ps = acc_ps.tile([B, JL, D], f32, tag="s_ps")
                for p in range(P):
                    for h in range(H):
                        nc.tensor.matmul(
                            s_ps[:],
                            usin[:, p, h, :],
                            stat[:, p, :, :, h],
                            start=(p == 0 and h == 0),
                            stop=(p == P - 1 and h == H - 1),
                        )
                return s_ps

            def squash(r, s_ps):
                """Evacuate s (with 1/Z or 1/N), squash -> vT fp32 [B,(jl,d)]."""
                sT = small.tile([B, JL, D], f32, tag="sT")
                if r == 0:
                    nc.scalar.activation(sT[:], s_ps[:], AF.Copy, scale=1.0 / N)
                else:
                    rz = recipz[:].unsqueeze(2).to_broadcast((B, JL, D))
                    nc.vector.tensor_tensor(sT[:], s_ps[:], rz, ALU.mult)
                sq = small.tile([B, JL], f32, tag="sq")
                s2 = small.tile([B, JL, D], f32, tag="s2")
                nc.vector.tensor_tensor(s2[:], sT[:], sT[:], ALU.mult)
                nc.vector.tensor_reduce(sq[:], s2[:], AX.X, ALU.add)
                # factor f = sq / ((1+sq) * sqrt(sq+EPS))
                sqe = small.tile([B, JL], f32, tag="sqe")
                nc.vector.tensor_scalar_add(sqe[:], sq[:], EPS)
                rt = small.tile([B, JL], f32, tag="rt")
                nc.scalar.activation(rt[:], sqe[:], AF.Sqrt)
                den = small.tile([B, JL], f32, tag="den")
                nc.vector.tensor_scalar_add(den[:], sq[:], 1.0)
                nc.vector.tensor_tensor(den[:], den[:], rt[:], ALU.mult)
                rec = small.tile([B, JL], f32, tag="rec")
                nc.vector.reciprocal(rec[:], den[:])
                fac = small.tile([B, JL], f32, tag="fac")
                nc.vector.tensor_tensor(fac[:], sq[:], rec[:], ALU.mult)
                vT = small.tile([B, JL, D], f32, tag="vT")
                fb = fac[:].unsqueeze(2).to_broadcast((B, JL, D))
                nc.vector.tensor_tensor(vT[:], sT[:], fb, ALU.mult)
                return vT

            def g_pass(vT):
                """g_psum[(jl,gc), k] = mean_b <u_hat, v> (Sel folds 1/B)."""
                for g in range(G):
                    nc.vector.tensor_copy(
                        vblk[32 * g : 32 * g + 32, 32 * g : 32 * g + 32].rearrange(
                            "b (j d) -> b j d", j=JL
                        ),
                        vT[:],
                    )
                for kc in range(K // KC):
                    g_ps = acc_ps.tile([2 * G, KC], f32, tag="g_ps")
                    for pp in range(P // 2):
                        rho = rho_ps.tile([128, 2, KC], f32, tag="rho")
                        for i in range(2):
                            p = 2 * pp + i
                            nc.tensor.matmul(
                                rho[:, i],
                                vblk[:],
                                u7in[:, p, kc * KC : (kc + 1) * KC],
                                start=True,
                                stop=True,
                            )
                        rh = prodp.tile([128, 2, KC], f16, tag="rh")
                        nc.scalar.activation(rh[:], rho[:], AF.Copy)
                        pr = prodp.tile([128, 2, KC], f16, tag="pr")
                        w8v = w8in[:, 2 * pp : 2 * pp + 2, kc * KC : (kc + 1) * KC]
                        nc.vector.tensor_tensor(pr[:], rh[:], w8v, ALU.mult)
                        for i in range(2):
                            nc.tensor.matmul(
                                g_ps[:],
                                selred[:],
                                pr[:, i],
                                start=(pp == 0 and i == 0),
                                stop=(pp == P // 2 - 1 and i == 1),
                            )
                    nc.vector.tensor_tensor(
                        b_sb[:, kc * KC : (kc + 1) * KC],
                        b_sb[:, kc * KC : (kc + 1) * KC],
                        g_ps[:],
                        ALU.add,
                    )
                    # relayout+exp this kc's logits while g continues:
                    # b[8, 128]-chunks -> PE transpose -> psum [128, 8]
                    # -> ACT exp -> cq[:, jl, h=16*gc + (4*kc+cc)]
                    for cc in range(4):
                        kk = 4 * kc + cc
                        tp = tiny_ps.tile([128, 2 * G], f16, tag="tp_ps")
                        nc.tensor.transpose(
                            tp[:],
                            b_sb[:, kk * 128 : (kk + 1) * 128],
                            ident8[:],
                        )
                        nc.scalar.activation(
                            cq[:, :, (4 * kc + cc) :: 4 * G].rearrange(
                                "q j g -> q (j g)"
                            ),
                            tp[:],
                            AF.Exp,
                        )

            def softmax_stage():
                """cq already holds exp(b) (filled during g_pass); compute
                recipz = 1/Z and cw = cq * wsin."""
                part = small.tile([128, JL], f32, tag="part")
                nc.vector.tensor_reduce(part[:], cq[:], AX.X, ALU.add)
                z_ps = tiny_ps.tile([1, JL], f32, tag="z_ps")
                nc.tensor.matmul(z_ps[:], ones128[:], part[:], start=True, stop=True)
                zs = small.tile([1, JL], f32, tag="zs")
                nc.vector.tensor_copy(zs[:], z_ps[:])
                zb_ps = tiny_ps.tile([B, JL], f32, tag="z_ps")
                nc.tensor.matmul(zb_ps[:], onesrow[:], zs[:], start=True, stop=True)
                nc.vector.reciprocal(recipz[:], zb_ps[:])
                # cw = cq (broadcast over p, d) * wsin
                cb = cq[:].unsqueeze(2).to_broadcast((128, JL, D, H))
                for p in range(P):
                    nc.vector.tensor_tensor(cw[:, p], wsin[:, p], cb, ALU.mult)

            # ---------------- the 3 routing iterations ----------------
            nreps = 1
            if variant.startswith("rep"):
                nreps, nr = int(variant[3:]), R
            else:
                nr = {"iter0": 1, "iter0g": 1, "iter1": 2, "full": R}[variant]
            vT = None
            for rep in range(nreps):
                if rep > 0:
                    nc.gpsimd.memset(b_sb[:], 0.0)
                for r in range(nr):
                    if r > 0:
                        softmax_stage()
                    s_ps = s_pass(r, wsin if r == 0 else cw)
                    vT = squash(r, s_ps)
                    if (r < nr - 1) or variant == "iter0g":
                        g_pass(vT)

            nc.sync.dma_start(out=vout_d.ap(), in_=vT[:])

    nc.compile()
    return nc


def _get_program(variant="full"):
    if variant not in _prog_cache:
        _prog_cache[variant] = _build_program(variant)
    return _prog_cache[variant]


def _host_constants():
    # Sel[(gc,jl,d), (jl',gc')] = 1/B if gc==gc' and jl==jl' else 0
    sel = np.zeros((G, JL, D, JL, G), dtype=np.float16)
    for g in range(G):
        for j in range(JL):
            sel[g, j, :, j, g] = 1.0 / B
    sel = sel.reshape(128, JL * G)
    ones = np.ones((128, 1), dtype=np.float32)
    onesrow = np.ones((1, B), dtype=np.float32)
    ident = np.eye(2 * G, dtype=np.float16)
    return sel, ones, onesrow, ident


def _prep_inputs(u, w):
    """u: (B, N, P) f32; w: (J, N, P, D) f32 -> per-core SBUF-ready arrays."""
    u16 = u.astype(np.float16)
    w16 = w.astype(np.float16)
    # usin[q, p, h, b] = u[b, 128h+q, p]
    usin = np.ascontiguousarray(
        u16.reshape(B, H, 128, P).transpose(2, 3, 1, 0)
    )
    # u7in[(gc,b), (p,k)] = u[b, 2048gc+k, p]  (fp8 — g-chain tolerance)
    import ml_dtypes
    f8np = ml_dtypes.float8_e4m3
    u7in = np.ascontiguousarray(
        u.reshape(B, G, K, P).transpose(1, 0, 3, 2).reshape(128, P, K)
    ).astype(f8np)
    wsins, w8ins = [], []
    for c in range(NCORES):
        wc = w16[c * JL : (c + 1) * JL]  # (JL, N, P, D)
        # wsin[q, p, jl, d, h] = w[jl, 128h+q, p, d]
        wsins.append(
            np.ascontiguousarray(
                wc.reshape(JL, H, 128, P, D).transpose(2, 3, 0, 4, 1)
            )
        )
        # w8in[(gc,jl,d), (p,k)] = w[jl, 2048gc+k, p, d]  (fp8)
        w8ins.append(
            np.ascontiguousarray(
                w[c * JL : (c + 1) * JL]
                .reshape(JL, G, K, P, D)
                .transpose(1, 0, 4, 3, 2)
                .reshape(128, P, K)
            ).astype(f8np)
        )
    return usin, u7in, wsins, w8ins


def _run(u_i, w_ij, trace=False, variant="full"):
    _ensure_path()
    from concourse.bass_utils import run_bass_kernel_spmd

    nc = _get_program(variant)
    sel, ones, onesrow, ident = _host_constants()
    u = np.ascontiguousarray(u_i, dtype=np.float32)[:, 0]  # (B, N, P)
    w = np.ascontiguousarray(w_ij[0], dtype=np.float32)  # (J, N, P, D)
    usin, u7in, wsins, w8ins = _prep_inputs(u, w)

    in_maps = [
        {
            "usin": usin,
            "wsin": wsins[c],
            "u7in": u7in,
            "w8in": w8ins[c],
            "selred": sel,
            "ones128": ones,
            "onesrow": onesrow,
            "ident8": ident,
        }
        for c in range(NCORES)
    ]
    res = run_bass_kernel_spmd(nc, in_maps, list(range(NCORES)), trace=trace)
    v = np.concatenate([res.results[c]["vout"] for c in range(NCORES)], axis=1)
    return v[:, :, None, :, None].astype(np.float32), res.exec_time_ns


def kernel(u_i: np.ndarray, w_ij: np.ndarray) -> np.ndarray:
    out, _ = _run(u_i, w_ij, trace=False)
    return out


def run_traced(u_i: np.ndarray, w_ij: np.ndarray):
    """Like kernel() but returns (output, exec_time_ns) via NTFF tracing.

    Falls back to untraced execution when the axon NTFF hook is missing.
    """
    try:
        return _run(u_i, w_ij, trace=True)
    except ModuleNotFoundError:
        return _run(u_i, w_ij, trace=False)
